# revision 16
# baseline (speedup 1.0000x reference)
"""TRN2 Bass kernel for nn_Der_SRec: attention-fused embedding scorer.

Math (per row b of batch B=16384, D=512):
  z,c,f = Ez[n[b]], Ec[n[b]], E[n[b]]       (per side u/v)
  s_z = a3 . relu(A2 @ relu(A1a @ z + A1f @ f + ab1) + ab2) + ab3
  s_c = same with c
  w_z = softmax([s_z, s_c])[0] = sigmoid(s_z - s_c)   (ab3 cancels)
  u = c + w_z * (z - c)
  h  = relu(bn(uv @ w1.T + b1));  out = h @ w2.T + b2  (bn folded into w1/b1)

Distribution: data-parallel over batch across 8 cores (2048 rows/core);
tables + weights replicated. On-chip: activations live in [feature, batch]
layout (feature on partitions) so the TensorE contracts features; the gather
produces [batch, feature] so each gathered tile is PE-transposed. The
indirect gather casts f32 tables to bf16 in the DMA; all matmuls run in bf16
with f32 PSUM accumulation; the shared `full`-conditioned first-layer term is
computed once per side and added to both scores' PSUM.
"""
import numpy as np
import ml_dtypes

import concourse.bass as bass
import concourse.mybir as mybir
import concourse.tile as tile
from concourse.bass_utils import run_bass_kernel_spmd
from concourse.masks import make_identity

P = 128
D = 512
DC = D // P          # feature chunks per 512
B = 16384
NCORES = 8
BC = B // NCORES     # rows per core (2048)
BT = 512             # batch tile (matmul N)
NBT = BC // BT       # batch tiles per core (4)
NSUB = BT // P       # gather subtiles per batch tile (4)
NU = 100000
NV = 50000
BN_EPS = 1e-5

f32 = mybir.dt.float32
bf16 = mybir.dt.bfloat16
i32 = mybir.dt.int32

_uid = [0]


def _split_multi_waits(nc):
    """walrus here encodes at most ONE sem wait per ISA instruction; Tile's
    sem assignment can emit several on one instruction (kernel-tail drain,
    matmuls with several producers). Hoist extras onto single-wait NoOps
    inserted just before, on the same engine stream (same-engine program
    order preserves semantics)."""
    for fn in nc.m.functions:
        for blk in fn.blocks:
            insts = blk.instructions
            i = 0
            while i < len(insts):
                inst = insts[i]
                si = inst.sync_info
                if si is not None and len(si.on_wait) > 1:
                    waits = list(si.on_wait)
                    for w in waits[:-1]:
                        _uid[0] += 1
                        nop = mybir.InstNoOp(
                            name=f"waitsplit_{_uid[0]}", ins=[], outs=[]
                        )
                        nop.engine = inst.engine
                        nop.sync_info = mybir.SyncInfo(on_wait=[w], on_update=[])
                        insts.insert(i, nop)
                        i += 1
                    inst.sync_info = mybir.SyncInfo(
                        on_wait=[waits[-1]], on_update=list(si.on_update)
                    )
                i += 1


def _build():
    nc = bass.Bass()

    tab_u = {
        "z": nc.dram_tensor("Ez_u", [NU, D], f32, kind="ExternalInput"),
        "c": nc.dram_tensor("Ec_u", [NU, D], f32, kind="ExternalInput"),
        "f": nc.dram_tensor("E_u", [NU, D], f32, kind="ExternalInput"),
    }
    tab_v = {
        "z": nc.dram_tensor("Ez_v", [NV, D], f32, kind="ExternalInput"),
        "c": nc.dram_tensor("Ec_v", [NV, D], f32, kind="ExternalInput"),
        "f": nc.dram_tensor("E_v", [NV, D], f32, kind="ExternalInput"),
    }
    nodes_u = nc.dram_tensor("nodes_u", [BC], i32, kind="ExternalInput")
    nodes_v = nc.dram_tensor("nodes_v", [BC], i32, kind="ExternalInput")

    # weight layout: [D_in, X] row-major in DRAM, loaded as [p, kc, X] in SBUF
    A1aT = nc.dram_tensor("A1aT", [D, D], bf16, kind="ExternalInput")
    A1fT = nc.dram_tensor("A1fT", [D, D], bf16, kind="ExternalInput")
    A2T = nc.dram_tensor("A2T", [D, D], bf16, kind="ExternalInput")
    W1uT = nc.dram_tensor("W1uT", [D, D], bf16, kind="ExternalInput")
    W1vT = nc.dram_tensor("W1vT", [D, D], bf16, kind="ExternalInput")
    a3p = nc.dram_tensor("a3p", [D], bf16, kind="ExternalInput")
    w2T = nc.dram_tensor("w2T", [D], bf16, kind="ExternalInput")
    ab1 = nc.dram_tensor("ab1", [DC, P], f32, kind="ExternalInput")
    ab2 = nc.dram_tensor("ab2", [DC, P], f32, kind="ExternalInput")
    bh = nc.dram_tensor("bh", [DC, P], f32, kind="ExternalInput")

    out = nc.dram_tensor("out", [BC], f32, kind="ExternalOutput")

    with tile.TileContext(nc) as tc:
        with (
            tc.tile_pool(name="const", bufs=1) as const,
            tc.tile_pool(name="rawp", bufs=30) as rawp,
            tc.tile_pool(name="xp", bufs=2) as xp,
            tc.tile_pool(name="hp", bufs=2) as hp,
            tc.tile_pool(name="sp", bufs=2) as sp,
            tc.tile_pool(name="ps_tr", bufs=2, space="PSUM") as ps_tr,
            tc.tile_pool(name="ps_mm", bufs=5, space="PSUM") as ps_mm,
            tc.tile_pool(name="ps_aux", bufs=1, space="PSUM") as ps_aux,
        ):
            ident = const.tile([P, P], bf16)
            make_identity(nc, ident)
            ones_bc = const.tile([1, P], bf16)
            nc.vector.memset(ones_bc[:], 1.0)

            # bt0 index columns first (unblocks the first gathers), on two
            # different HWDGE queues; the rest loads behind them.
            idx_u = const.tile([P, BC // P], i32)
            idx_v = const.tile([P, BC // P], i32)
            nodes_u_pt = nodes_u[:].rearrange("(t p) -> p t", p=P)
            nodes_v_pt = nodes_v[:].rearrange("(t p) -> p t", p=P)
            nc.sync.dma_start(out=idx_u[:, 0:NSUB], in_=nodes_u_pt[:, 0:NSUB])
            nc.scalar.dma_start(out=idx_v[:, 0:NSUB], in_=nodes_v_pt[:, 0:NSUB])
            nc.sync.dma_start(out=idx_u[:, NSUB:], in_=nodes_u_pt[:, NSUB:])
            nc.scalar.dma_start(out=idx_v[:, NSUB:], in_=nodes_v_pt[:, NSUB:])


            def load_w(dram):
                t = const.tile([P, DC, D], bf16, name=f"w_{dram.name}")
                nc.sync.dma_start(
                    out=t[:], in_=dram[:, :].rearrange("(kc p) m -> p kc m", p=P)
                )
                return t

            A1aT_sb = load_w(A1aT)
            A1fT_sb = load_w(A1fT)
            A2T_sb = load_w(A2T)
            W1uT_sb = load_w(W1uT)
            W1vT_sb = load_w(W1vT)

            def load_vec(dram, dt):
                t = const.tile([P, DC], dt, name=f"v_{dram.name}")
                nc.sync.dma_start(
                    out=t[:], in_=dram[:].rearrange("(kc p) -> p kc", p=P)
                )
                return t

            a3p_sb = load_vec(a3p, bf16)
            w2T_sb = load_vec(w2T, bf16)

            def load_bias(dram):
                t = const.tile([P, DC], f32, name=f"b_{dram.name}")
                nc.sync.dma_start(
                    out=t[:], in_=dram[:, :].rearrange("kc p -> p kc")
                )
                return t

            ab1_sb = load_bias(ab1)
            ab2_sb = load_bias(ab2)
            bh_sb = load_bias(bh)

            def stage_gather(bt):
                """Issue the 24 indirect row-gathers for batch tile bt."""
                raws = {}
                for side, tabs, idx in (("u", tab_u, idx_u), ("v", tab_v, idx_v)):
                    for kind in ("z", "c", "f"):
                        rs = []
                        for s in range(NSUB):
                            raw = rawp.tile(
                                [P, D], bf16, name=f"raw_{side}{kind}{s}", tag="raw"
                            )
                            nc.gpsimd.indirect_dma_start(
                                out=raw[:],
                                out_offset=None,
                                in_=tabs[kind][:],
                                in_offset=bass.IndirectOffsetOnAxis(
                                    ap=idx[:, bt * NSUB + s : bt * NSUB + s + 1],
                                    axis=0,
                                ),
                            )
                            rs.append(raw)
                        raws[(side, kind)] = rs
                return raws

            def stage_transpose(raws):
                """PE-transpose gathered [batch, feat] tiles into [feat, batch]."""
                xT = {}
                for key, rs in raws.items():
                    side, kind = key
                    x = xp.tile(
                        [P, DC, BT], bf16, name=f"xT_{side}{kind}",
                        tag=f"xT_{side}{kind}",
                    )
                    for c in range(DC):
                        pst = ps_tr.tile(
                            [P, BT], bf16, name=f"pst{c}", tag="pst"
                        )
                        for s in range(NSUB):
                            nc.tensor.transpose(
                                pst[:, s * P : (s + 1) * P],
                                rs[s][:, c * P : (c + 1) * P],
                                ident[:],
                            )
                        nc.any.tensor_copy(x[:, c, :], pst[:])
                    xT[key] = x
                return xT

            raws_cur = stage_gather(0)
            for bt in range(NBT):
                xT = stage_transpose(raws_cur)
                if bt + 1 < NBT:
                    raws_cur = stage_gather(bt + 1)

                # ---- per-side attention fusion -> u_t, v_t bf16 [p, kc, BT]
                fused = {}
                for side in ("u", "v"):
                    xz, xc, xf = (
                        xT[(side, "z")], xT[(side, "c")], xT[(side, "f")],
                    )

                    def mlp_layer(wa, xa, bias_sb, name, add_sb=None):
                        h = hp.tile(
                            [P, DC, BT], bf16, name=f"h_{name}", tag=f"h_{name}"
                        )
                        for m in range(DC):
                            ps = ps_mm.tile(
                                [P, BT], f32, name=f"ps_{name}{m}", tag="mm"
                            )
                            for k in range(DC):
                                nc.tensor.matmul(
                                    ps[:],
                                    wa[:, k, m * P : (m + 1) * P],
                                    xa[:, k, :],
                                    start=(k == 0),
                                    stop=(k == DC - 1),
                                )
                            if add_sb is not None:
                                nc.vector.tensor_add(ps[:], ps[:], add_sb[:, m, :])
                            nc.scalar.activation(
                                out=h[:, m, :],
                                in_=ps[:],
                                func=mybir.ActivationFunctionType.Relu,
                                bias=bias_sb[:, m : m + 1],
                                scale=1.0,
                            )
                        return h

                    # shared first-layer term from `full`: hf = A1f.T @ f
                    hf = hp.tile([P, DC, BT], f32, name=f"hf_{side}", tag="hf")
                    for m in range(DC):
                        ps = ps_mm.tile([P, BT], f32, name=f"ps_hf{m}", tag="mm")
                        for k in range(DC):
                            nc.tensor.matmul(
                                ps[:],
                                A1fT_sb[:, k, m * P : (m + 1) * P],
                                xf[:, k, :],
                                start=(k == 0),
                                stop=(k == DC - 1),
                            )
                        nc.vector.tensor_copy(hf[:, m, :], ps[:])

                    h1z = mlp_layer(A1aT_sb, xz, ab1_sb, "1z", add_sb=hf)
                    h1c = mlp_layer(A1aT_sb, xc, ab1_sb, "1c", add_sb=hf)
                    h2z = mlp_layer(A2T_sb, h1z, ab2_sb, "2z")
                    h2c = mlp_layer(A2T_sb, h1c, ab2_sb, "2c")

                    # d = s_z - s_c = a3 . (h2z - h2c)  [1, BT]
                    hd = hp.tile([P, DC, BT], bf16, name=f"hd_{side}", tag="hd")
                    nc.vector.tensor_sub(hd[:], h2z[:], h2c[:])
                    dps = ps_aux.tile([1, BT], f32, name="dps", tag="aux")
                    for k in range(DC):
                        nc.tensor.matmul(
                            dps[:],
                            a3p_sb[:, k : k + 1],
                            hd[:, k, :],
                            start=(k == 0),
                            stop=(k == DC - 1),
                        )
                    wz = sp.tile([1, BT], bf16, name="wz", tag="wz")
                    nc.scalar.activation(
                        out=wz[:],
                        in_=dps[:],
                        func=mybir.ActivationFunctionType.Sigmoid,
                    )
                    # broadcast wz across partitions via K=1 ones-matmul
                    wbc = ps_aux.tile([P, BT], f32, name="wbc", tag="aux")
                    nc.tensor.matmul(
                        wbc[:], ones_bc[:], wz[:], start=True, stop=True
                    )
                    # fused = c + wz * (z - c)
                    zmc = hp.tile(
                        [P, DC, BT], bf16, name=f"zmc_{side}", tag="zmc"
                    )
                    nc.vector.tensor_sub(zmc[:], xz[:], xc[:])
                    uf = hp.tile(
                        [P, DC, BT], bf16, name=f"fused_{side}", tag=f"fused_{side}"
                    )
                    # per-chunk so head matmuls can start on early chunks
                    for k in range(DC):
                        nc.vector.tensor_tensor(
                            out=zmc[:, k, :], in0=zmc[:, k, :], in1=wbc[:],
                            op=mybir.AluOpType.mult,
                        )
                        nc.vector.tensor_add(uf[:, k, :], zmc[:, k, :], xc[:, k, :])
                    fused[side] = uf

                # ---- head: h = relu(W1u.T@u + W1v.T@v + bh) ; out = w2.h + b2
                hh = hp.tile([P, DC, BT], bf16, name="hh", tag="hh")
                for m in range(DC):
                    ps = ps_mm.tile([P, BT], f32, name=f"ps_hh{m}", tag="mm")
                    for k in range(DC):
                        nc.tensor.matmul(
                            ps[:],
                            W1uT_sb[:, k, m * P : (m + 1) * P],
                            fused["u"][:, k, :],
                            start=(k == 0),
                            stop=False,
                        )
                    for k in range(DC):
                        nc.tensor.matmul(
                            ps[:],
                            W1vT_sb[:, k, m * P : (m + 1) * P],
                            fused["v"][:, k, :],
                            start=False,
                            stop=(k == DC - 1),
                        )
                    nc.scalar.activation(
                        out=hh[:, m, :],
                        in_=ps[:],
                        func=mybir.ActivationFunctionType.Relu,
                        bias=bh_sb[:, m : m + 1],
                        scale=1.0,
                    )
                ops = ps_aux.tile([1, BT], f32, name="ops", tag="aux")
                for k in range(DC):
                    nc.tensor.matmul(
                        ops[:],
                        w2T_sb[:, k : k + 1],
                        hh[:, k, :],
                        start=(k == 0),
                        stop=(k == DC - 1),
                    )
                osb = sp.tile([1, BT], f32, name="osb", tag="osb")
                nc.scalar.activation(
                    out=osb[:],
                    in_=ops[:],
                    func=mybir.ActivationFunctionType.Copy,
                )
                nc.sync.dma_start(
                    out=out[bt * BT : (bt + 1) * BT].unsqueeze(0), in_=osb[:]
                )

    _split_multi_waits(nc)
    return nc


_NC_CACHE = None


def _get_nc():
    global _NC_CACHE
    if _NC_CACHE is None:
        _NC_CACHE = _build()
    return _NC_CACHE


def _prep_host(inputs):
    """Host-side weight preprocessing shared by all cores."""
    f = lambda k: np.asarray(inputs[k], np.float32)
    att_w1 = f("att_w1")
    att_w2 = f("att_w2")
    att_w3 = f("att_w3")
    w1 = f("w1")
    s = f("bn_gamma") / np.sqrt(f("bn_var") + BN_EPS)
    t = f("bn_beta") - f("bn_mean") * s
    bf = lambda a: np.ascontiguousarray(a).astype(ml_dtypes.bfloat16)
    common = {
        "Ez_u": f("Ez_u"), "Ec_u": f("Ec_u"), "E_u": f("E_u"),
        "Ez_v": f("Ez_v"), "Ec_v": f("Ec_v"), "E_v": f("E_v"),
        "A1aT": bf(att_w1[:, :D].T),
        "A1fT": bf(att_w1[:, D:].T),
        "A2T": bf(att_w2.T),
        "W1uT": bf((w1[:, :D] * s[:, None]).T),
        "W1vT": bf((w1[:, D:] * s[:, None]).T),
        "a3p": bf(att_w3[0]),
        "w2T": bf(f("w2")[0]),
        "ab1": np.ascontiguousarray(f("att_b1").reshape(DC, P)),
        "ab2": np.ascontiguousarray(f("att_b2").reshape(DC, P)),
        "bh": np.ascontiguousarray((f("b1") * s + t).reshape(DC, P)),
    }
    return common


def kernel(**inputs):
    common = _prep_host(inputs)
    nodes_u = np.asarray(inputs["nodes_u"]).astype(np.int32)
    nodes_v = np.asarray(inputs["nodes_v"]).astype(np.int32)

    in_maps = []
    for i in range(NCORES):
        m = dict(common)
        m["nodes_u"] = np.ascontiguousarray(nodes_u[i * BC : (i + 1) * BC])
        m["nodes_v"] = np.ascontiguousarray(nodes_v[i * BC : (i + 1) * BC])
        in_maps.append(m)

    nc = _get_nc()
    res = run_bass_kernel_spmd(nc, in_maps, core_ids=list(range(NCORES)))
    out = np.concatenate([np.asarray(r["out"]) for r in res.results])
    return (out + np.float32(np.asarray(inputs["b2"]).reshape(-1)[0])).astype(np.float32)
